# revision 26
# baseline (speedup 1.0000x reference)
"""Multi-head attention layer (B=4, S=2048, H=8, D=128) on 8 trn2 NeuronCores.

Sharding: core c handles batch b = c//2 and query half c%2 (1024 query rows).
Each core computes K/V over all 2048 keys of its batch, its 1024x2048 slice of
the attention matrix for all 8 heads, and its slice of the final
residual+LayerNorm output.  Outputs are assembled on the host by pure
concatenation - no cross-core reduction.

Precision: the score/value matmul pipeline runs in fp16 (Q/K/V/Wo operands,
exp output, attention weights), which bounds the relative error near 1e-3.
The attention-probability output tensor, the residual, and the LayerNorm are
computed in fp32.  The big probability transpose (for attn @ V) rides the
DMA XBAR transpose engine instead of the tensor engine.
"""

import math
from contextlib import ExitStack

import numpy as np

import concourse.bass as bass
import concourse.mybir as mybir
import concourse.tile as tile
from concourse import bacc
from concourse.bass_utils import run_bass_kernel_spmd
from concourse.masks import make_identity

B, S, D, H = 4, 2048, 128, 8
QS = S // 2            # 1024 query rows per core
NQT = QS // 128        # 8 query tiles per core
NKT = S // 128         # 16 key tiles
HG = 2                 # head groups (SBUF capacity)
HPG = H // HG          # 4 heads per group
SCALE = 1.0 / math.sqrt(D)
LN_EPS = 1e-6

FP32 = mybir.dt.float32
FP16 = mybir.dt.float16
I32 = mybir.dt.int32
AF = mybir.ActivationFunctionType
OP = mybir.AluOpType


def build_bass():
    nc = bacc.Bacc(trn_type="TRN2")

    enc_b = nc.dram_tensor("enc_b", [S, D], FP32, kind="ExternalInput")
    enc_q = nc.dram_tensor("enc_q", [QS, D], FP32, kind="ExternalInput")
    mask_p = nc.dram_tensor("mask_p", [QS, S], I32, kind="ExternalInput")
    wq_d = nc.dram_tensor("wq", [D, H * D], FP32, kind="ExternalInput")
    wk_d = nc.dram_tensor("wk", [D, H * D], FP32, kind="ExternalInput")
    wv_d = nc.dram_tensor("wv", [D, H * D], FP32, kind="ExternalInput")
    wo_d = nc.dram_tensor("wo", [H * D, D], FP32, kind="ExternalInput")
    lns_d = nc.dram_tensor("ln_scale", [D], FP32, kind="ExternalInput")
    lnb_d = nc.dram_tensor("ln_bias", [D], FP32, kind="ExternalInput")
    attn_o = nc.dram_tensor("attn_o", [H, QS, S], FP32, kind="ExternalOutput")
    out_o = nc.dram_tensor("out_o", [QS, D], FP32, kind="ExternalOutput")

    with TileKernel(nc) as tk:
        tk.run(enc_b, enc_q, mask_p, wq_d, wk_d, wv_d, wo_d, lns_d, lnb_d,
               attn_o, out_o)
    nc.finalize()
    return nc


class TileKernel:
    def __init__(self, nc):
        self.nc = nc
        self.ctx = ExitStack()
        self.tc = None

    def __enter__(self):
        self.tc = self.ctx.enter_context(tile.TileContext(self.nc))
        return self

    def __exit__(self, *exc):
        return self.ctx.__exit__(*exc)

    def run(self, enc_b, enc_q, mask_p, wq_d, wk_d, wv_d, wo_d, lns_d, lnb_d,
            attn_o, out_o):
        nc, tc, ctx = self.nc, self.tc, self.ctx

        consts = ctx.enter_context(tc.tile_pool(name="consts", bufs=1))

        # ---- weights (fp32 load + fp16 working copies) ----
        wq_sb = consts.tile([D, H * D], FP32)
        wk_sb = consts.tile([D, H * D], FP32)
        wv_sb = consts.tile([D, H * D], FP32)
        wo_sb = consts.tile([D, H * D], FP32)
        nc.sync.dma_start(wq_sb, wq_d[:, :])
        nc.sync.dma_start(wk_sb, wk_d[:, :])
        nc.sync.dma_start(wv_sb, wv_d[:, :])
        # wo in [d, (h m)] layout: block h holds Wo_h = wo[h*D:(h+1)*D, :]
        nc.sync.dma_start(
            wo_sb.rearrange("p (h m) -> p h m", h=H),
            wo_d.rearrange("(h d) m -> d h m", h=H),
        )
        wq16 = consts.tile([D, H * D], FP16)
        wk16 = consts.tile([D, H * D], FP16)
        wv16 = consts.tile([D, H * D], FP16)
        wo16 = consts.tile([D, H * D], FP16)
        nc.vector.tensor_copy(wq16, wq_sb)
        nc.vector.tensor_copy(wk16, wk_sb)
        nc.vector.tensor_copy(wv16, wv_sb)
        nc.vector.tensor_copy(wo16, wo_sb)
        # ln params broadcast along partitions
        lns_sb = consts.tile([128, D], FP32)
        lnb_sb = consts.tile([128, D], FP32)
        nc.sync.dma_start(lns_sb, bass.AP(lns_d, 0, [[0, 128], [1, D]]))
        nc.sync.dma_start(lnb_sb, bass.AP(lnb_d, 0, [[0, 128], [1, D]]))

        # ---- enc loads + transposes ----
        ident = consts.tile([128, 128], FP32)
        make_identity(nc, ident)
        encq_sb = consts.tile([128, NQT * D], FP32)  # tile t = enc_q[t*128:+128,:]
        nc.sync.dma_start(
            encq_sb.rearrange("p (t d) -> p t d", t=NQT),
            enc_q.rearrange("(t p) d -> p t d", p=128),
        )
        encT_sb = consts.tile([D, S], FP16)    # enc_b transposed  [d, seq]
        encqT_sb = consts.tile([D, QS], FP16)  # enc_q transposed  [d, q]
        acc_sb = consts.tile([128, NQT * D], FP32)   # sum_h attn@V@Wo, [q, d]
        zero_bias = consts.tile([128, 1], FP32)
        eps_bias = consts.tile([128, 1], FP32)
        nc.vector.memset(zero_bias, 0.0)
        nc.vector.memset(eps_bias, LN_EPS)
        self._zero_bias = zero_bias
        mu_all = consts.tile([128, NQT], FP32)
        var_all = consts.tile([128, NQT], FP32)
        std_all = consts.tile([128, NQT], FP32)
        rstd_all = consts.tile([128, NQT], FP32)
        nc.vector.memset(acc_sb, 0.0)

        with tc.tile_pool(name="encnat", bufs=1) as encnat_pool, \
             tc.tile_pool(name="tp0", bufs=2, space="PSUM") as tp0:
            enc_nat = encnat_pool.tile([128, NKT * D], FP32)
            nc.sync.dma_start(
                enc_nat.rearrange("p (t d) -> p t d", t=NKT),
                enc_b.rearrange("(t p) d -> p t d", p=128),
            )
            for t in range(NKT):
                ps = tp0.tile([128, D], FP32, tag="tp0")
                nc.tensor.transpose(ps, enc_nat[:, t * D:(t + 1) * D], ident)
                nc.scalar.copy(encT_sb[:, t * 128:(t + 1) * 128], ps)
            for t in range(NQT):
                ps = tp0.tile([128, D], FP32, tag="tp0")
                nc.tensor.transpose(ps, encq_sb[:, t * D:(t + 1) * D], ident)
                nc.scalar.copy(encqT_sb[:, t * 128:(t + 1) * 128], ps)

        # ---- long-lived working pools ----
        mask_pool = ctx.enter_context(tc.tile_pool(name="mask", bufs=4))
        p_pool = ctx.enter_context(tc.tile_pool(name="p", bufs=4))
        af_pool = ctx.enter_context(tc.tile_pool(name="af", bufs=3))
        attnT_pool = ctx.enter_context(tc.tile_pool(name="attnT", bufs=2))
        outw_pool = ctx.enter_context(tc.tile_pool(name="outw", bufs=2))
        stat_pool = ctx.enter_context(tc.tile_pool(name="stat", bufs=8))

        for g in range(HG):
            with tc.tile_pool(name=f"grp{g}", bufs=1) as grp_pool:
                kt_sb = grp_pool.tile([128, HPG * S], FP16)   # [d,(hl k)]
                v_sb = grp_pool.tile([128, NKT * 512], FP16)  # [k,(kt hl d)]
                # -- projections for this head group --
                with tc.tile_pool(name=f"projps{g}", bufs=2,
                                  space="PSUM") as proj_ps:
                    for hl in range(HPG):
                        h = g * HPG + hl
                        ps = proj_ps.tile([128, S], FP32, tag="proj")
                        for c in range(S // 512):
                            nc.tensor.matmul(
                                ps[:, c * 512:(c + 1) * 512],
                                wk16[:, h * D:(h + 1) * D],
                                encT_sb[:, c * 512:(c + 1) * 512],
                                start=True, stop=True)
                        nc.scalar.copy(kt_sb[:, hl * S:(hl + 1) * S], ps)
                    for c4 in range(NKT // 4):
                        ps = proj_ps.tile([128, S], FP32, tag="proj")
                        for j in range(4):
                            kt = c4 * 4 + j
                            nc.tensor.matmul(
                                ps[:, j * 512:(j + 1) * 512],
                                encT_sb[:, kt * D:(kt + 1) * D],
                                wv16[:, g * 512:(g + 1) * 512],
                                start=True, stop=True)
                        nc.scalar.copy(v_sb[:, c4 * 2048:(c4 + 1) * 2048], ps)

                with tc.tile_pool(name=f"qth{g}", bufs=2) as qth_pool, \
                     tc.tile_pool(name=f"sps{g}", bufs=2, space="PSUM") as s_ps, \
                     tc.tile_pool(name=f"pvps{g}", bufs=2, space="PSUM") as pv_ps, \
                     tc.tile_pool(name=f"wops{g}", bufs=2, space="PSUM") as wo_ps:
                    for half in range(2):
                        # Q^T for this (group, half): [d, (hl q512)]
                        qth_sb = qth_pool.tile([D, HPG * 512], FP16, tag="qth")
                        for c in range(2):
                            qps = s_ps.tile([128, 1024], FP32, tag="s")
                            for j in range(2):
                                hl = c * 2 + j
                                h = g * HPG + hl
                                nc.tensor.matmul(
                                    qps[:, j * 512:(j + 1) * 512],
                                    wq16[:, h * D:(h + 1) * D],
                                    encqT_sb[:, half * 512:(half + 1) * 512],
                                    start=True, stop=True)
                            nc.scalar.copy(
                                qth_sb[:, c * 1024:(c + 1) * 1024], qps)

                        m_tiles = []
                        for qtl in range(4):
                            qt = half * 4 + qtl
                            m_t = mask_pool.tile([128, S], FP16, tag="mask")
                            # gpsimd DMA casts int32 {0,1} -> fp16 in flight
                            nc.gpsimd.dma_start(
                                m_t, mask_p[qt * 128:(qt + 1) * 128, :])
                            m_tiles.append(m_t)

                        for hl in range(HPG):
                            h = g * HPG + hl
                            attnT_sb = attnT_pool.tile(
                                [128, NKT * 512], FP16, tag="attnT")
                            attnT_r = attnT_sb.rearrange(
                                "p (kt q) -> p kt q", kt=NKT)
                            for qtl in range(4):
                                qt = half * 4 + qtl
                                self.attention_tile(
                                    s_ps, p_pool, af_pool, stat_pool,
                                    qth_sb, kt_sb, m_tiles[qtl], attnT_r,
                                    attn_o, h, hl, qt, qtl)
                            # -- PV: out_h^T[d, q512] accumulated over kt --
                            pv_t = pv_ps.tile([128, 512], FP32, tag="pv")
                            for kt in range(NKT):
                                nc.tensor.matmul(
                                    pv_t,
                                    v_sb[:, kt * 512 + hl * D:
                                         kt * 512 + (hl + 1) * D],
                                    attnT_sb[:, kt * 512:(kt + 1) * 512],
                                    start=(kt == 0), stop=(kt == NKT - 1))
                            outw_t = outw_pool.tile([128, 512], FP16,
                                                    tag="outw")
                            nc.scalar.copy(outw_t, pv_t)
                            # -- Wo: out_pre[q, dm] for 4 q-tiles --
                            wo_t = wo_ps.tile([128, 512], FP32, tag="wo")
                            for i in range(4):
                                nc.tensor.matmul(
                                    wo_t[:, i * 128:(i + 1) * 128],
                                    outw_t[:, i * 128:(i + 1) * 128],
                                    wo16[:, h * D:(h + 1) * D],
                                    start=True, stop=True)
                            dst = acc_sb[:, half * 512:(half + 1) * 512]
                            nc.vector.tensor_add(dst, dst, wo_t)

        # ---- residual + LayerNorm ----
        junk_pool = ctx.enter_context(tc.tile_pool(name="junk", bufs=2))
        for qt in range(NQT):
            x = acc_sb[:, qt * D:(qt + 1) * D]
            sum_t = stat_pool.tile([128, 1], FP32, tag="lnsum")
            nc.vector.scalar_tensor_tensor(
                out=x, in0=x, scalar=0.0, in1=encq_sb[:, qt * D:(qt + 1) * D],
                op0=OP.add, op1=OP.add, accum_out=sum_t)
            nc.vector.tensor_scalar_mul(
                mu_all[:, qt:qt + 1], sum_t, 1.0 / D)
            nc.vector.tensor_scalar(
                out=x, in0=x, scalar1=mu_all[:, qt:qt + 1], scalar2=None,
                op0=OP.subtract)
            sq_t = junk_pool.tile([128, D], FP32, tag="junk")
            nc.vector.scalar_tensor_tensor(
                out=sq_t, in0=x, scalar=1.0, in1=x,
                op0=OP.mult, op1=OP.mult, accum_out=var_all[:, qt:qt + 1])
        # var_all holds sum(xc^2); sqrt(sum/D + eps) via ACT's free affine
        nc.scalar.activation(std_all, var_all, AF.Sqrt, bias=eps_bias,
                             scale=1.0 / D)
        nc.vector.reciprocal(rstd_all, std_all)
        for qt in range(NQT):
            x = acc_sb[:, qt * D:(qt + 1) * D]
            nc.vector.tensor_scalar(
                out=x, in0=x, scalar1=rstd_all[:, qt:qt + 1], scalar2=None,
                op0=OP.mult)
            nc.vector.tensor_mul(x, x, lns_sb)
            nc.vector.tensor_add(x, x, lnb_sb)
        nc.sync.dma_start(
            out_o.rearrange("(t p) d -> p t d", p=128),
            acc_sb.rearrange("p (t d) -> p t d", t=NQT),
        )

    def attention_tile(self, s_ps, p_pool, af_pool, stat_pool,
                       qth_sb, kt_sb, m_t, attnT_r, attn_o, h, hl, qt, qtl):
        """softmax row block: scores -> exp(fp16) -> mask+rowsum ->
        normalize in place -> f32 DMA out + XBAR transpose into attnT."""
        nc = self.nc
        p_t = p_pool.tile([128, S], FP16, tag="p")
        lhs = qth_sb[:, hl * 512 + qtl * 128: hl * 512 + (qtl + 1) * 128]
        for c in range(2):
            s_t = s_ps.tile([128, 1024], FP32, tag="s")
            for j in range(2):
                nc.tensor.matmul(
                    s_t[:, j * 512:(j + 1) * 512],
                    lhs,
                    kt_sb[:, hl * S + (c * 2 + j) * 512:
                          hl * S + (c * 2 + j + 1) * 512],
                    start=True, stop=True)
            nc.scalar.activation(
                p_t[:, c * 1024:(c + 1) * 1024], s_t, AF.Exp,
                bias=self._zero_bias, scale=SCALE)
        rs_t = stat_pool.tile([128, 1], FP32, tag="rs")
        nc.vector.scalar_tensor_tensor(
            out=p_t, in0=p_t, scalar=1.0, in1=m_t,
            op0=OP.mult, op1=OP.mult, accum_out=rs_t)
        rinv_t = stat_pool.tile([128, 1], FP32, tag="rinv")
        nc.vector.reciprocal(rinv_t, rs_t)
        nc.vector.tensor_scalar(
            out=p_t, in0=p_t, scalar1=rinv_t, scalar2=None, op0=OP.mult)
        attn_f = af_pool.tile([128, S], FP32, tag="af")
        nc.vector.tensor_copy(attn_f, p_t)
        nc.sync.dma_start(attn_o[h, qt * 128:(qt + 1) * 128, :], attn_f)
        nc.sync.dma_start_transpose(
            attnT_r[:, :, qtl * 128:(qtl + 1) * 128], p_t)


# ---------------------------------------------------------------------------
_NC_CACHE = None


def _get_nc():
    global _NC_CACHE
    if _NC_CACHE is None:
        _NC_CACHE = build_bass()
    return _NC_CACHE


def make_in_maps(inputs):
    enc = np.asarray(inputs["enc"], np.float32)
    mask = np.asarray(inputs["mask"], np.int32)
    wq = np.asarray(inputs["Wq"], np.float32)
    wk = np.asarray(inputs["Wk"], np.float32)
    wv = np.asarray(inputs["Wv"], np.float32)
    wo = np.asarray(inputs["Wo"], np.float32)
    lns = np.asarray(inputs["ln_scale"], np.float32)
    lnb = np.asarray(inputs["ln_bias"], np.float32)

    in_maps = []
    for c in range(8):
        b, half = divmod(c, 2)
        q0 = half * QS
        in_maps.append({
            "enc_b": np.ascontiguousarray(enc[b]),
            "enc_q": np.ascontiguousarray(enc[b, q0:q0 + QS]),
            "mask_p": np.ascontiguousarray(mask[b, 0, q0:q0 + QS, :]),
            "wq": wq, "wk": wk, "wv": wv, "wo": wo,
            "ln_scale": lns, "ln_bias": lnb,
        })
    return in_maps


def kernel(**inputs):
    nc = _get_nc()
    in_maps = make_in_maps(inputs)
    res = run_bass_kernel_spmd(nc, in_maps, core_ids=list(range(8)))
    attn = np.empty((B, H, S, S), np.float32)
    out = np.empty((B, S, D), np.float32)
    for c in range(8):
        b, half = divmod(c, 2)
        q0 = half * QS
        attn[b, :, q0:q0 + QS, :] = res.results[c]["attn_o"]
        out[b, q0:q0 + QS, :] = res.results[c]["out_o"]
    return out, attn
